# revision 26
# baseline (speedup 1.0000x reference)
"""DiffAttention kernel for 8 TRN2 NeuronCores (Bass/Tile).

Reference computation (see problem): x [1,128,32,32,32] is stride-2
subsampled to xs [128, N=4096 tokens]; qkv = w_qkv @ xs per head
(4 heads, head_dim 32, split into two halves of 16 for the two
softmaxes); diff_attn = softmax(q1k1) - 0.1*softmax(q2k2); out = diff
attn @ v, reshaped back to [1,128,16,16,16].

Sharding: tensor-parallel over (head, query-half) = 8 shards, one per
core. Each core computes its head's full K/V over all 4096 tokens and
attention for its 2048 queries.

Per-core dataflow (all on-chip, flash-style, no NxN HBM traffic):
  - k1,k2 / q1,q2 projections via PE with col-tiled placement so that
    strip 1 (partitions 32:48) holds the (q1,k1) pair and strip 2
    (partitions 64:80) holds (q2,k2); scores are computed TRANSPOSED,
    sT[m,n] = k^T q, so the softmax denominator can be folded into the
    AV matmul via a ones-column appended to v^T (no extra PE streams).
  - exp on ACT directly from PSUM (scale folded into the activation).
  - AV: out^T[d,n] accumulated over m-chunks in PSUM; AV1 at psum
    partitions 0:33, AV2 at 64:97 (col-tiled, run concurrently).
  - finalize: PE-transpose av -> [n,33], per-partition reciprocal of
    the sum column, combine out = av1/s1 - 0.1*av2/s2 on DVE.
"""

import numpy as np
import ml_dtypes

import concourse.bass as bass
import concourse.mybir as mybir
import concourse.tile as tile
from concourse import bacc
from concourse.bass import ts, ds
from concourse.bass_utils import run_bass_kernel_spmd

BF16 = mybir.dt.bfloat16
F32 = mybir.dt.float32
NP_BF16 = ml_dtypes.bfloat16

C = 128          # channels
HEADS = 4
HD = 32          # head_dim
DH = 16          # d_half
LAMBDA = 0.1
SCALE = HD ** -0.5
R = 2
N_CORES = 8
N = 4096         # tokens after subsample
NQ = N // 2      # queries per core

# weight tensor column layout (w input, [128, 96]):
WV = slice(0, 32)     # w_v^T   (rhs of vT matmuls)
WK1 = slice(32, 48)   # w_k1^T
WK2 = slice(48, 64)   # w_k2^T
WQ1 = slice(64, 80)   # w_q1^T
WQ2 = slice(80, 96)   # w_q2^T


def build_nc(NT=N, NQL=NQ, NBS=1024):
    """Build the SPMD Bass program for one core = (head, query-half).

    Per-core inputs:
      xs    [128, NT]   bf16  all tokens, channel-major (for K and V)
      xq    [128, NQL]  bf16  this core's query tokens
      w     [128, 96]   bf16  columns per WV/WK1/WK2/WQ1/WQ2 slices
      ident [128, 33]   f32   identity blocks at partitions 0:33, 64:97
    Output:
      out   [NQL, 32]   f32   attention output (n, d) for the queries
    """
    assert NT % 512 == 0 and NQL % NBS == 0 and NBS % 512 == 0
    assert NQL % 128 == 0
    MC = NT // 128        # m-chunks (key/value chunks of 128 tokens)
    NC128 = NQL // 128    # query chunks of 128 for the finalize
    Exp = mybir.ActivationFunctionType.Exp

    nc = bacc.Bacc()
    xs_d = nc.declare_dram_parameter("xs", [C, NT], BF16, isOutput=False)
    xq_d = nc.declare_dram_parameter("xq", [C, NQL], BF16, isOutput=False)
    w_d = nc.declare_dram_parameter("w", [C, 96], BF16, isOutput=False)
    id_d = nc.declare_dram_parameter("ident", [C, 33], F32, isOutput=False)
    out_d = nc.declare_dram_parameter("out", [NQL, HD], F32, isOutput=True)

    with tile.TileContext(nc) as tc:
        with (
            tc.tile_pool(name="consts", bufs=1) as consts,
            tc.tile_pool(name="mains", bufs=1) as mains,
        ):
            # input DMAs spread across trigger queues (each dma_start costs
            # ~1.4us on its issuing engine): weights first so the projection
            # matmuls can start as soon as the first xs chunk lands
            w_sb = consts.tile([C, 96], BF16)
            nc.sync.dma_start(out=w_sb[:, :], in_=w_d[:, :])
            id_sb = consts.tile([C, 33], F32)
            nc.sync.dma_start(out=id_sb[:, :], in_=id_d[:, :])
            def chunked_dma(eng, dst, src, total):
                # small leading chunks so dependent matmuls start early,
                # then larger ones to amortize the per-dma_start cost
                sizes, rem = [], total
                for sz in (512, 512, 1024):
                    if rem >= sz:
                        sizes.append(sz)
                        rem -= sz
                while rem > 0:
                    sz = 2048 if rem >= 2048 else 512
                    sizes.append(sz)
                    rem -= sz
                off = 0
                for sz in sizes:
                    eng.dma_start(out=dst[:, ds(off, sz)],
                                  in_=src[:, ds(off, sz)])
                    off += sz

            xs_sb = mains.tile([C, NT], BF16)
            chunked_dma(nc.gpsimd, xs_sb, xs_d, NT)
            xq_sb = mains.tile([C, NQL], BF16)
            chunked_dma(nc.scalar, xq_sb, xq_d, NQL)

            kk_sb = mains.tile([C, NT], BF16)    # parts 32:48 k1, 64:80 k2
            qq_sb = mains.tile([C, NQL], BF16)   # parts 32:48 q1, 64:80 q2
            vTa_sb = mains.tile([C, MC * 33], BF16)  # per chunk: v^T | ones
            av_sb = mains.tile([C, NQL], F32)    # parts 0:33 AV1|s1, 64:97 AV2|s2
            out_sb = mains.tile([C, NC128 * HD], F32)

            nc.vector.memset(vTa_sb[:, :], 1.0)

            # --- main attention loop; the k/q/vT projections are
            # interleaved into the first n-block's m-loop so the scalar
            # engine (the pacer) starts exp-ing within a few microseconds.
            # PSUM budget: sj 2x2 banks + av 2 banks + aux 2 banks = 8.
            with (
                tc.tile_pool(name="s_ps", bufs=2, space="PSUM") as spool,
                tc.tile_pool(name="av_ps", bufs=1, space="PSUM") as avpool,
                tc.tile_pool(name="aux_ps", bufs=2, space="PSUM") as auxpool,
                tc.tile_pool(name="e_sb", bufs=3) as epool,
                tc.tile_pool(name="fin_sb", bufs=2) as fsb,
            ):
                CQ_NB = NBS // 128   # query chunks per n-block

                def project_q(t):
                    ps_q = auxpool.tile([C, 512], F32, tag="aux", name="ps_q")
                    nc.tensor.matmul(ps_q[32:48, :], lhsT=w_sb[:, WQ1],
                                     rhs=xq_sb[:, ts(t, 512)], start=True, stop=True)
                    nc.tensor.matmul(ps_q[64:80, :], lhsT=w_sb[:, WQ2],
                                     rhs=xq_sb[:, ts(t, 512)], start=True, stop=True)
                    nc.vector.tensor_copy(qq_sb[32:48, ts(t, 512)], ps_q[32:48, :])
                    nc.vector.tensor_copy(qq_sb[64:80, ts(t, 512)], ps_q[64:80, :])

                def project_kv(t):
                    ps_kv = auxpool.tile([C, 512], F32, tag="aux", name="ps_kv")
                    nc.tensor.matmul(ps_kv[32:48, :], lhsT=w_sb[:, WK1],
                                     rhs=xs_sb[:, ts(t, 512)], start=True, stop=True)
                    nc.tensor.matmul(ps_kv[64:80, :], lhsT=w_sb[:, WK2],
                                     rhs=xs_sb[:, ts(t, 512)], start=True, stop=True)
                    nc.vector.tensor_copy(kk_sb[32:48, ts(t, 512)], ps_kv[32:48, :])
                    nc.vector.tensor_copy(kk_sb[64:80, ts(t, 512)], ps_kv[64:80, :])

                def project_vt(m):
                    ps_vt = auxpool.tile([C, 512], F32, tag="aux", name="ps_vt")
                    nc.tensor.matmul(ps_vt[:, 0:HD], lhsT=xs_sb[:, ts(m, 128)],
                                     rhs=w_sb[:, WV], start=True, stop=True)
                    nc.vector.tensor_copy(vTa_sb[:, ds(m * 33, HD)], ps_vt[:, 0:HD])

                def finalize_nb(nb):
                    # transpose av -> [n, 33] (av1/av2 pairs are in distinct
                    # row groups so they can run concurrently), then
                    # normalize by the sums column and combine on DVE
                    psT1 = auxpool.tile([C, CQ_NB * 64], F32, tag="aux")
                    psT2 = auxpool.tile([C, CQ_NB * 64], F32, tag="aux")
                    for cq in range(CQ_NB):
                        gq = nb * CQ_NB + cq
                        nc.tensor.transpose(psT1[:, ds(cq * 64, 33)],
                                            av_sb[0:33, ts(gq, 128)], id_sb[0:33, :])
                        nc.tensor.transpose(psT2[:, ds(cq * 64, 33)],
                                            av_sb[64:97, ts(gq, 128)], id_sb[64:97, :])
                    r1_sb = fsb.tile([C, CQ_NB], F32, tag="r1")
                    r2_sb = fsb.tile([C, CQ_NB], F32, tag="r2")
                    sum1_view = psT1[:, :].rearrange("p (c x) -> p c x", x=64)[:, :, 32:33]
                    sum2_view = psT2[:, :].rearrange("p (c x) -> p c x", x=64)[:, :, 32:33]
                    nc.vector.reciprocal(r1_sb[:, :, None], sum1_view)
                    nc.vector.reciprocal(r2_sb[:, :, None], sum2_view)
                    nc.vector.tensor_scalar_mul(r2_sb[:, :], r2_sb[:, :], -LAMBDA)
                    o1_sb = fsb.tile([C, CQ_NB * HD], F32, tag="o1")
                    o2_sb = fsb.tile([C, CQ_NB * HD], F32, tag="o2")
                    av1t_view = psT1[:, :].rearrange("p (c x) -> p c x", x=64)[:, :, 0:32]
                    av2t_view = psT2[:, :].rearrange("p (c x) -> p c x", x=64)[:, :, 0:32]
                    o1_view = o1_sb[:, :].rearrange("p (c d) -> p c d", d=HD)
                    o2_view = o2_sb[:, :].rearrange("p (c d) -> p c d", d=HD)
                    nc.vector.tensor_tensor(
                        o1_view, av1t_view,
                        r1_sb[:, :, None].to_broadcast((C, CQ_NB, HD)),
                        mybir.AluOpType.mult)
                    nc.vector.tensor_tensor(
                        o2_view, av2t_view,
                        r2_sb[:, :, None].to_broadcast((C, CQ_NB, HD)),
                        mybir.AluOpType.mult)
                    nc.vector.tensor_tensor(
                        out_sb[:, ds(nb * CQ_NB * HD, CQ_NB * HD)],
                        o1_sb[:, :], o2_sb[:, :], mybir.AluOpType.add)

                pending_finalize = None
                N_NB = NQL // NBS
                for nb in range(N_NB):
                    av_ps = avpool.tile([C, NBS], F32, tag="av")
                    if nb == 0:
                        # all query chunks THIS block reads, before its scores
                        for t in range(NBS // 512):
                            project_q(t)
                    for m in range(MC):
                        if nb == 0:
                            if m % 4 == 0:
                                project_kv(m // 4)
                            project_vt(m)
                            # later blocks' query chunks, spread off the
                            # first scores' critical path
                            if m % 4 == 2:
                                t = NBS // 512 + (m - 2) // 4
                                if t < NQL // 512:
                                    project_q(t)
                        if m == 2 and pending_finalize is not None:
                            pending_finalize()
                            pending_finalize = None
                        first, last = (m == 0), (m == MC - 1)
                        for j in range(NBS // 512):
                            nsl = ds(nb * NBS + j * 512, 512)
                            # s1 and s2 share one psum tile (adjacent banks):
                            # both row-tiled matmuls are released by the same
                            # ACT read, so the scheduler keeps them adjacent
                            # and they run concurrently in strips 1 and 2.
                            sj_ps = spool.tile([C, 1024], F32, tag="sj")
                            nc.tensor.matmul(sj_ps[:, 0:512],
                                             lhsT=kk_sb[32:48, ts(m, 128)],
                                             rhs=qq_sb[32:48, nsl],
                                             start=True, stop=True)
                            nc.tensor.matmul(sj_ps[:, 512:1024],
                                             lhsT=kk_sb[64:80, ts(m, 128)],
                                             rhs=qq_sb[64:80, nsl],
                                             start=True, stop=True)
                            # one exp over both matrices' scores
                            e_sb = epool.tile([C, 1024], BF16, tag="e")
                            nc.scalar.activation(e_sb[:, :], sj_ps[:, :], Exp,
                                                 scale=SCALE)
                            # av1 (parts 0:33) and av2 (64:97) share psum
                            # banks on disjoint partitions; the sim's group
                            # check is partition-unaware, hence the skip.
                            nc.tensor.matmul(av_ps[0:33, ts(j, 512)],
                                             lhsT=vTa_sb[:, ds(m * 33, 33)],
                                             rhs=e_sb[:, 0:512],
                                             start=first, stop=last,
                                             skip_group_check=True)
                            nc.tensor.matmul(av_ps[64:97, ts(j, 512)],
                                             lhsT=vTa_sb[:, ds(m * 33, 33)],
                                             rhs=e_sb[:, 512:1024],
                                             start=first, stop=last,
                                             skip_group_check=True)
                    # drain the accumulators; only the last block (the tail,
                    # when ACT is idle) borrows the scalar engine
                    nc.vector.tensor_copy(av_sb[0:33, ds(nb * NBS, NBS)], av_ps[0:33, :])
                    if nb == N_NB - 1:
                        nc.scalar.copy(av_sb[64:97, ds(nb * NBS, NBS)], av_ps[64:97, :])
                    else:
                        nc.vector.tensor_copy(av_sb[64:97, ds(nb * NBS, NBS)], av_ps[64:97, :])
                    if nb == N_NB - 1:
                        finalize_nb(nb)
                    else:
                        pending_finalize = (lambda nb=nb: finalize_nb(nb))
                if pending_finalize is not None:
                    pending_finalize()

            nc.sync.dma_start(
                out=out_d[:, :].rearrange("(c p) d -> p c d", p=C),
                in_=out_sb[:, :].rearrange("p (c d) -> p c d", d=HD),
            )
    nc.compile()
    return nc


def make_identity_input():
    ident = np.zeros((C, 33), np.float32)
    ident[0:33, :] = np.eye(33, dtype=np.float32)
    ident[64:97, :] = np.eye(33, dtype=np.float32)
    return ident


def make_in_maps(x, w_qkv):
    """Host-side sharding: subsample, pack per-core inputs."""
    xs = np.ascontiguousarray(x[0][:, ::R, ::R, ::R]).reshape(C, N)
    xs_b = xs.astype(NP_BF16)
    ident = make_identity_input()
    in_maps = []
    for core in range(N_CORES):
        h, half = divmod(core, 2)
        wq = w_qkv[h * 96: h * 96 + 32]       # [32, 128]
        wk = w_qkv[h * 96 + 32: h * 96 + 64]
        wv = w_qkv[h * 96 + 64: h * 96 + 96]
        w = np.empty((C, 96), np.float32)
        w[:, WV] = wv.T
        w[:, WK1] = wk[0:DH].T
        w[:, WK2] = wk[DH:HD].T
        w[:, WQ1] = wq[0:DH].T
        w[:, WQ2] = wq[DH:HD].T
        in_maps.append({
            "xs": xs_b,
            "xq": np.ascontiguousarray(xs_b[:, half * NQ:(half + 1) * NQ]),
            "w": w.astype(NP_BF16),
            "ident": ident,
        })
    return in_maps


_NC_CACHE = {}


def get_nc():
    if "nc" not in _NC_CACHE:
        _NC_CACHE["nc"] = build_nc()
    return _NC_CACHE["nc"]


LAST_RESULTS = None  # BassKernelResults of the most recent kernel() call


def kernel(x, w_qkv, trace=False, **trace_kwargs):
    global LAST_RESULTS
    x = np.asarray(x)
    w_qkv = np.asarray(w_qkv)
    in_maps = make_in_maps(x, w_qkv)
    nc = get_nc()
    res = run_bass_kernel_spmd(nc, in_maps, list(range(N_CORES)),
                               trace=trace, **trace_kwargs)
    LAST_RESULTS = res
    out_hnd = np.empty((HEADS, N, HD), np.float32)
    for core in range(N_CORES):
        h, half = divmod(core, 2)
        out_hnd[h, half * NQ:(half + 1) * NQ, :] = res.results[core]["out"]
    return out_hnd.reshape(1, C, 16, 16, 16)


# revision 27
# speedup vs baseline: 1.0122x; 1.0122x over previous
"""DiffAttention kernel for 8 TRN2 NeuronCores (Bass/Tile).

Reference computation (see problem): x [1,128,32,32,32] is stride-2
subsampled to xs [128, N=4096 tokens]; qkv = w_qkv @ xs per head
(4 heads, head_dim 32, split into two halves of 16 for the two
softmaxes); diff_attn = softmax(q1k1) - 0.1*softmax(q2k2); out = diff
attn @ v, reshaped back to [1,128,16,16,16].

Sharding: tensor-parallel over (head, query-half) = 8 shards, one per
core. Each core computes its head's full K/V over all 4096 tokens and
attention for its 2048 queries.

Per-core dataflow (all on-chip, flash-style, no NxN HBM traffic):
  - k1,k2 / q1,q2 projections via PE with col-tiled placement so that
    strip 1 (partitions 32:48) holds the (q1,k1) pair and strip 2
    (partitions 64:80) holds (q2,k2); scores are computed TRANSPOSED,
    sT[m,n] = k^T q, so the softmax denominator can be folded into the
    AV matmul via a ones-column appended to v^T (no extra PE streams).
  - exp on ACT directly from PSUM (scale folded into the activation).
  - AV: out^T[d,n] accumulated over m-chunks in PSUM; AV1 at psum
    partitions 0:33, AV2 at 64:97 (col-tiled, run concurrently).
  - finalize: PE-transpose av -> [n,33], per-partition reciprocal of
    the sum column, combine out = av1/s1 - 0.1*av2/s2 on DVE.
"""

import numpy as np
import ml_dtypes

import concourse.bass as bass
import concourse.mybir as mybir
import concourse.tile as tile
from concourse import bacc
from concourse.bass import ts, ds
from concourse.bass_utils import run_bass_kernel_spmd

BF16 = mybir.dt.bfloat16
F32 = mybir.dt.float32
NP_BF16 = ml_dtypes.bfloat16

C = 128          # channels
HEADS = 4
HD = 32          # head_dim
DH = 16          # d_half
LAMBDA = 0.1
SCALE = HD ** -0.5
R = 2
N_CORES = 8
N = 4096         # tokens after subsample
NQ = N // 2      # queries per core

# weight tensor column layout (w input, [128, 96]):
WV = slice(0, 32)     # w_v^T   (rhs of vT matmuls)
WK1 = slice(32, 48)   # w_k1^T
WK2 = slice(48, 64)   # w_k2^T
WQ1 = slice(64, 80)   # w_q1^T
WQ2 = slice(80, 96)   # w_q2^T


def build_nc(NT=N, NQL=NQ, NBS=1024):
    """Build the SPMD Bass program for one core = (head, query-half).

    Per-core inputs:
      xs    [128, NT]   bf16  all tokens, channel-major (for K and V)
      xq    [128, NQL]  bf16  this core's query tokens
      w     [128, 96]   bf16  columns per WV/WK1/WK2/WQ1/WQ2 slices
      ident [128, 33]   f32   identity blocks at partitions 0:33, 64:97
    Output:
      out   [NQL, 32]   f32   attention output (n, d) for the queries
    """
    assert NT % 512 == 0 and NQL % NBS == 0 and NBS % 512 == 0
    assert NQL % 128 == 0
    MC = NT // 128        # m-chunks (key/value chunks of 128 tokens)
    NC128 = NQL // 128    # query chunks of 128 for the finalize
    Exp = mybir.ActivationFunctionType.Exp

    nc = bacc.Bacc()
    xs_d = nc.declare_dram_parameter("xs", [C, NT], BF16, isOutput=False)
    xq_d = nc.declare_dram_parameter("xq", [C, NQL], BF16, isOutput=False)
    w_d = nc.declare_dram_parameter("w", [C, 96], BF16, isOutput=False)
    id_d = nc.declare_dram_parameter("ident", [C, 33], F32, isOutput=False)
    out_d = nc.declare_dram_parameter("out", [NQL, HD], F32, isOutput=True)

    with tile.TileContext(nc) as tc:
        with (
            tc.tile_pool(name="consts", bufs=1) as consts,
            tc.tile_pool(name="mains", bufs=1) as mains,
        ):
            # input DMAs spread across trigger queues (each dma_start costs
            # ~1.4us on its issuing engine): weights first so the projection
            # matmuls can start as soon as the first xs chunk lands
            w_sb = consts.tile([C, 96], BF16)
            nc.sync.dma_start(out=w_sb[:, :], in_=w_d[:, :])
            id_sb = consts.tile([C, 33], F32)
            nc.sync.dma_start(out=id_sb[:, :], in_=id_d[:, :])
            def chunked_dma(eng, dst, src, total):
                # small leading chunks so dependent matmuls start early,
                # then larger ones to amortize the per-dma_start cost
                sizes, rem = [], total
                for sz in (512, 512, 1024):
                    if rem >= sz:
                        sizes.append(sz)
                        rem -= sz
                while rem > 0:
                    sz = 2048 if rem >= 2048 else 512
                    sizes.append(sz)
                    rem -= sz
                off = 0
                for sz in sizes:
                    eng.dma_start(out=dst[:, ds(off, sz)],
                                  in_=src[:, ds(off, sz)])
                    off += sz

            xs_sb = mains.tile([C, NT], BF16)
            chunked_dma(nc.gpsimd, xs_sb, xs_d, NT)
            xq_sb = mains.tile([C, NQL], BF16)
            chunked_dma(nc.scalar, xq_sb, xq_d, NQL)

            kk_sb = mains.tile([C, NT], BF16)    # parts 32:48 k1, 64:80 k2
            qq_sb = mains.tile([C, NQL], BF16)   # parts 32:48 q1, 64:80 q2
            vTa_sb = mains.tile([C, MC * 33], BF16)  # per chunk: v^T | ones
            av_sb = mains.tile([C, NQL], F32)    # parts 0:33 AV1|s1, 64:97 AV2|s2
            out_sb = mains.tile([C, NC128 * HD], F32)

            nc.vector.memset(vTa_sb[:, :], 1.0)

            # --- main attention loop; the k/q/vT projections are
            # interleaved into the first n-block's m-loop so the scalar
            # engine (the pacer) starts exp-ing within a few microseconds.
            # PSUM budget: sj 2x2 banks + av 2 banks + aux 2 banks = 8.
            with (
                tc.tile_pool(name="s_ps", bufs=2, space="PSUM") as spool,
                tc.tile_pool(name="av_ps", bufs=1, space="PSUM") as avpool,
                tc.tile_pool(name="aux_ps", bufs=2, space="PSUM") as auxpool,
                tc.tile_pool(name="e_sb", bufs=3) as epool,
                tc.tile_pool(name="fin_sb", bufs=2) as fsb,
            ):
                CQ_NB = NBS // 128   # query chunks per n-block

                def project_q(t):
                    ps_q = auxpool.tile([C, 512], F32, tag="aux", name="ps_q")
                    nc.tensor.matmul(ps_q[32:48, :], lhsT=w_sb[:, WQ1],
                                     rhs=xq_sb[:, ts(t, 512)], start=True, stop=True)
                    nc.tensor.matmul(ps_q[64:80, :], lhsT=w_sb[:, WQ2],
                                     rhs=xq_sb[:, ts(t, 512)], start=True, stop=True)
                    nc.vector.tensor_copy(qq_sb[32:48, ts(t, 512)], ps_q[32:48, :])
                    nc.vector.tensor_copy(qq_sb[64:80, ts(t, 512)], ps_q[64:80, :])

                def project_kv(t):
                    ps_kv = auxpool.tile([C, 512], F32, tag="aux", name="ps_kv")
                    nc.tensor.matmul(ps_kv[32:48, :], lhsT=w_sb[:, WK1],
                                     rhs=xs_sb[:, ts(t, 512)], start=True, stop=True)
                    nc.tensor.matmul(ps_kv[64:80, :], lhsT=w_sb[:, WK2],
                                     rhs=xs_sb[:, ts(t, 512)], start=True, stop=True)
                    nc.vector.tensor_copy(kk_sb[32:48, ts(t, 512)], ps_kv[32:48, :])
                    nc.vector.tensor_copy(kk_sb[64:80, ts(t, 512)], ps_kv[64:80, :])

                def project_vt(m):
                    ps_vt = auxpool.tile([C, 512], F32, tag="aux", name="ps_vt")
                    nc.tensor.matmul(ps_vt[:, 0:HD], lhsT=xs_sb[:, ts(m, 128)],
                                     rhs=w_sb[:, WV], start=True, stop=True)
                    nc.vector.tensor_copy(vTa_sb[:, ds(m * 33, HD)], ps_vt[:, 0:HD])

                def finalize_nb(nb):
                    # transpose av -> [n, 33] (av1/av2 pairs are in distinct
                    # row groups so they can run concurrently), then
                    # normalize by the sums column and combine on DVE
                    psT1 = auxpool.tile([C, CQ_NB * 64], F32, tag="aux")
                    psT2 = auxpool.tile([C, CQ_NB * 64], F32, tag="aux")
                    for cq in range(CQ_NB):
                        gq = nb * CQ_NB + cq
                        nc.tensor.transpose(psT1[:, ds(cq * 64, 33)],
                                            av_sb[0:33, ts(gq, 128)], id_sb[0:33, :])
                        nc.tensor.transpose(psT2[:, ds(cq * 64, 33)],
                                            av_sb[64:97, ts(gq, 128)], id_sb[64:97, :])
                    r1_sb = fsb.tile([C, CQ_NB], F32, tag="r1")
                    r2_sb = fsb.tile([C, CQ_NB], F32, tag="r2")
                    sum1_view = psT1[:, :].rearrange("p (c x) -> p c x", x=64)[:, :, 32:33]
                    sum2_view = psT2[:, :].rearrange("p (c x) -> p c x", x=64)[:, :, 32:33]
                    nc.vector.reciprocal(r1_sb[:, :, None], sum1_view)
                    nc.vector.reciprocal(r2_sb[:, :, None], sum2_view)
                    nc.vector.tensor_scalar_mul(r2_sb[:, :], r2_sb[:, :], -LAMBDA)
                    o1_sb = fsb.tile([C, CQ_NB * HD], F32, tag="o1")
                    o2_sb = fsb.tile([C, CQ_NB * HD], F32, tag="o2")
                    av1t_view = psT1[:, :].rearrange("p (c x) -> p c x", x=64)[:, :, 0:32]
                    av2t_view = psT2[:, :].rearrange("p (c x) -> p c x", x=64)[:, :, 0:32]
                    o1_view = o1_sb[:, :].rearrange("p (c d) -> p c d", d=HD)
                    o2_view = o2_sb[:, :].rearrange("p (c d) -> p c d", d=HD)
                    nc.vector.tensor_tensor(
                        o1_view, av1t_view,
                        r1_sb[:, :, None].to_broadcast((C, CQ_NB, HD)),
                        mybir.AluOpType.mult)
                    nc.vector.tensor_tensor(
                        o2_view, av2t_view,
                        r2_sb[:, :, None].to_broadcast((C, CQ_NB, HD)),
                        mybir.AluOpType.mult)
                    nc.vector.tensor_tensor(
                        out_sb[:, ds(nb * CQ_NB * HD, CQ_NB * HD)],
                        o1_sb[:, :], o2_sb[:, :], mybir.AluOpType.add)

                pending_finalize = None
                N_NB = NQL // NBS
                for nb in range(N_NB):
                    av_ps = avpool.tile([C, NBS], F32, tag="av")
                    if nb == 0:
                        # minimal chain to the first exp: q chunk 0 then the
                        # first k chunk, then the rest of this block's queries
                        project_q(0)
                        project_kv(0)
                        for t in range(1, NBS // 512):
                            project_q(t)
                    for m in range(MC):
                        if nb == 0:
                            project_vt(m)
                            # k chunks 3 iterations ahead of first use, on
                            # the off-beat of the scores they unblock
                            if m % 4 == 1 and (m - 1) // 4 + 1 < NT // 512:
                                project_kv((m - 1) // 4 + 1)
                            # later blocks' query chunks
                            if m % 4 == 3:
                                t = NBS // 512 + (m - 3) // 4
                                if t < NQL // 512:
                                    project_q(t)
                        if m == 2 and pending_finalize is not None:
                            pending_finalize()
                            pending_finalize = None
                        first, last = (m == 0), (m == MC - 1)
                        for j in range(NBS // 512):
                            nsl = ds(nb * NBS + j * 512, 512)
                            # s1 and s2 share one psum tile (adjacent banks):
                            # both row-tiled matmuls are released by the same
                            # ACT read, so the scheduler keeps them adjacent
                            # and they run concurrently in strips 1 and 2.
                            sj_ps = spool.tile([C, 1024], F32, tag="sj")
                            nc.tensor.matmul(sj_ps[:, 0:512],
                                             lhsT=kk_sb[32:48, ts(m, 128)],
                                             rhs=qq_sb[32:48, nsl],
                                             start=True, stop=True)
                            nc.tensor.matmul(sj_ps[:, 512:1024],
                                             lhsT=kk_sb[64:80, ts(m, 128)],
                                             rhs=qq_sb[64:80, nsl],
                                             start=True, stop=True)
                            # one exp over both matrices' scores
                            e_sb = epool.tile([C, 1024], BF16, tag="e")
                            nc.scalar.activation(e_sb[:, :], sj_ps[:, :], Exp,
                                                 scale=SCALE)
                            # av1 (parts 0:33) and av2 (64:97) share psum
                            # banks on disjoint partitions; the sim's group
                            # check is partition-unaware, hence the skip.
                            nc.tensor.matmul(av_ps[0:33, ts(j, 512)],
                                             lhsT=vTa_sb[:, ds(m * 33, 33)],
                                             rhs=e_sb[:, 0:512],
                                             start=first, stop=last,
                                             skip_group_check=True)
                            nc.tensor.matmul(av_ps[64:97, ts(j, 512)],
                                             lhsT=vTa_sb[:, ds(m * 33, 33)],
                                             rhs=e_sb[:, 512:1024],
                                             start=first, stop=last,
                                             skip_group_check=True)
                    # drain the accumulators; only the last block (the tail,
                    # when ACT is idle) borrows the scalar engine
                    nc.vector.tensor_copy(av_sb[0:33, ds(nb * NBS, NBS)], av_ps[0:33, :])
                    if nb == N_NB - 1:
                        nc.scalar.copy(av_sb[64:97, ds(nb * NBS, NBS)], av_ps[64:97, :])
                    else:
                        nc.vector.tensor_copy(av_sb[64:97, ds(nb * NBS, NBS)], av_ps[64:97, :])
                    if nb == N_NB - 1:
                        finalize_nb(nb)
                    else:
                        pending_finalize = (lambda nb=nb: finalize_nb(nb))
                if pending_finalize is not None:
                    pending_finalize()

            nc.sync.dma_start(
                out=out_d[:, :].rearrange("(c p) d -> p c d", p=C),
                in_=out_sb[:, :].rearrange("p (c d) -> p c d", d=HD),
            )
    nc.compile()
    return nc


def make_identity_input():
    ident = np.zeros((C, 33), np.float32)
    ident[0:33, :] = np.eye(33, dtype=np.float32)
    ident[64:97, :] = np.eye(33, dtype=np.float32)
    return ident


def make_in_maps(x, w_qkv):
    """Host-side sharding: subsample, pack per-core inputs."""
    xs = np.ascontiguousarray(x[0][:, ::R, ::R, ::R]).reshape(C, N)
    xs_b = xs.astype(NP_BF16)
    ident = make_identity_input()
    in_maps = []
    for core in range(N_CORES):
        h, half = divmod(core, 2)
        wq = w_qkv[h * 96: h * 96 + 32]       # [32, 128]
        wk = w_qkv[h * 96 + 32: h * 96 + 64]
        wv = w_qkv[h * 96 + 64: h * 96 + 96]
        w = np.empty((C, 96), np.float32)
        w[:, WV] = wv.T
        w[:, WK1] = wk[0:DH].T
        w[:, WK2] = wk[DH:HD].T
        w[:, WQ1] = wq[0:DH].T
        w[:, WQ2] = wq[DH:HD].T
        in_maps.append({
            "xs": xs_b,
            "xq": np.ascontiguousarray(xs_b[:, half * NQ:(half + 1) * NQ]),
            "w": w.astype(NP_BF16),
            "ident": ident,
        })
    return in_maps


_NC_CACHE = {}


def get_nc():
    if "nc" not in _NC_CACHE:
        _NC_CACHE["nc"] = build_nc()
    return _NC_CACHE["nc"]


LAST_RESULTS = None  # BassKernelResults of the most recent kernel() call


def kernel(x, w_qkv, trace=False, **trace_kwargs):
    global LAST_RESULTS
    x = np.asarray(x)
    w_qkv = np.asarray(w_qkv)
    in_maps = make_in_maps(x, w_qkv)
    nc = get_nc()
    res = run_bass_kernel_spmd(nc, in_maps, list(range(N_CORES)),
                               trace=trace, **trace_kwargs)
    LAST_RESULTS = res
    out_hnd = np.empty((HEADS, N, HD), np.float32)
    for core in range(N_CORES):
        h, half = divmod(core, 2)
        out_hnd[h, half * NQ:(half + 1) * NQ, :] = res.results[core]["out"]
    return out_hnd.reshape(1, C, 16, 16, 16)


# revision 28
# speedup vs baseline: 1.0202x; 1.0079x over previous
"""DiffAttention kernel for 8 TRN2 NeuronCores (Bass/Tile).

Reference computation (see problem): x [1,128,32,32,32] is stride-2
subsampled to xs [128, N=4096 tokens]; qkv = w_qkv @ xs per head
(4 heads, head_dim 32, split into two halves of 16 for the two
softmaxes); diff_attn = softmax(q1k1) - 0.1*softmax(q2k2); out = diff
attn @ v, reshaped back to [1,128,16,16,16].

Sharding: tensor-parallel over (head, query-half) = 8 shards, one per
core. Each core computes its head's full K/V over all 4096 tokens and
attention for its 2048 queries.

Per-core dataflow (all on-chip, flash-style, no NxN HBM traffic):
  - k1,k2 / q1,q2 projections via PE with col-tiled placement so that
    strip 1 (partitions 32:48) holds the (q1,k1) pair and strip 2
    (partitions 64:80) holds (q2,k2); scores are computed TRANSPOSED,
    sT[m,n] = k^T q, so the softmax denominator can be folded into the
    AV matmul via a ones-column appended to v^T (no extra PE streams).
  - exp on ACT directly from PSUM (scale folded into the activation).
  - AV: out^T[d,n] accumulated over m-chunks in PSUM; AV1 at psum
    partitions 0:33, AV2 at 64:97 (col-tiled, run concurrently).
  - finalize: PE-transpose av -> [n,33], per-partition reciprocal of
    the sum column, combine out = av1/s1 - 0.1*av2/s2 on DVE.
"""

import numpy as np
import ml_dtypes

import concourse.bass as bass
import concourse.mybir as mybir
import concourse.tile as tile
from concourse import bacc
from concourse.bass import ts, ds
from concourse.bass_utils import run_bass_kernel_spmd

BF16 = mybir.dt.bfloat16
F32 = mybir.dt.float32
NP_BF16 = ml_dtypes.bfloat16

C = 128          # channels
HEADS = 4
HD = 32          # head_dim
DH = 16          # d_half
LAMBDA = 0.1
SCALE = HD ** -0.5
R = 2
N_CORES = 8
N = 4096         # tokens after subsample
NQ = N // 2      # queries per core

# weight tensor column layout (w input, [128, 96]):
WV = slice(0, 32)     # w_v^T   (rhs of vT matmuls)
WK1 = slice(32, 48)   # w_k1^T
WK2 = slice(48, 64)   # w_k2^T
WQ1 = slice(64, 80)   # w_q1^T
WQ2 = slice(80, 96)   # w_q2^T


def build_nc(NT=N, NQL=NQ, NBS=1024):
    """Build the SPMD Bass program for one core = (head, query-half).

    Per-core inputs:
      xs    [128, NT]   bf16  all tokens, channel-major (for K and V)
      xq    [128, NQL]  bf16  this core's query tokens
      w     [128, 96]   bf16  columns per WV/WK1/WK2/WQ1/WQ2 slices
      ident [128, 33]   f32   identity blocks at partitions 0:33, 64:97
    Output:
      out   [NQL, 32]   f32   attention output (n, d) for the queries
    """
    assert NT % 512 == 0 and NQL % NBS == 0 and NBS % 512 == 0
    assert NQL % 128 == 0
    MC = NT // 128        # m-chunks (key/value chunks of 128 tokens)
    NC128 = NQL // 128    # query chunks of 128 for the finalize
    Exp = mybir.ActivationFunctionType.Exp

    nc = bacc.Bacc()
    xs_d = nc.declare_dram_parameter("xs", [C, NT], BF16, isOutput=False)
    xq_d = nc.declare_dram_parameter("xq", [C, NQL], BF16, isOutput=False)
    w_d = nc.declare_dram_parameter("w", [C, 96], BF16, isOutput=False)
    id_d = nc.declare_dram_parameter("ident", [C, 33], F32, isOutput=False)
    out_d = nc.declare_dram_parameter("out", [NQL, HD], F32, isOutput=True)

    with tile.TileContext(nc) as tc:
        with (
            tc.tile_pool(name="consts", bufs=1) as consts,
            tc.tile_pool(name="mains", bufs=1) as mains,
        ):
            # input DMAs spread across trigger queues (each dma_start costs
            # ~1.4us on its issuing engine): weights first so the projection
            # matmuls can start as soon as the first xs chunk lands
            w_sb = consts.tile([C, 96], BF16)
            nc.sync.dma_start(out=w_sb[:, :], in_=w_d[:, :])
            id_sb = consts.tile([C, 33], F32)
            nc.sync.dma_start(out=id_sb[:, :], in_=id_d[:, :])
            def chunked_dma(eng, dst, src, total):
                # small leading chunks so dependent matmuls start early,
                # then larger ones to amortize the per-dma_start cost
                sizes, rem = [], total
                for sz in (512, 512, 1024):
                    if rem >= sz:
                        sizes.append(sz)
                        rem -= sz
                while rem > 0:
                    sz = 2048 if rem >= 2048 else 512
                    sizes.append(sz)
                    rem -= sz
                off = 0
                for sz in sizes:
                    eng.dma_start(out=dst[:, ds(off, sz)],
                                  in_=src[:, ds(off, sz)])
                    off += sz

            xs_sb = mains.tile([C, NT], BF16)
            chunked_dma(nc.gpsimd, xs_sb, xs_d, NT)
            xq_sb = mains.tile([C, NQL], BF16)
            chunked_dma(nc.scalar, xq_sb, xq_d, NQL)

            kk_sb = mains.tile([C, NT], BF16)    # parts 32:48 k1, 64:80 k2
            qq_sb = mains.tile([C, NQL], BF16)   # parts 32:48 q1, 64:80 q2
            vTa_sb = mains.tile([C, MC * 33], BF16)  # per chunk: v^T | ones
            av_sb = mains.tile([C, NQL], F32)    # parts 0:33 AV1|s1, 64:97 AV2|s2
            out_sb = mains.tile([C, NC128 * HD], F32)

            nc.vector.memset(vTa_sb[:, :], 1.0)

            # --- main attention loop; the k/q/vT projections are
            # interleaved into the first n-block's m-loop so the scalar
            # engine (the pacer) starts exp-ing within a few microseconds.
            # PSUM budget: sj 2x2 banks + av 2 banks + aux 2 banks = 8.
            with (
                tc.tile_pool(name="s_ps", bufs=2, space="PSUM") as spool,
                tc.tile_pool(name="av_ps", bufs=1, space="PSUM") as avpool,
                tc.tile_pool(name="aux_ps", bufs=2, space="PSUM") as auxpool,
                tc.tile_pool(name="e_sb", bufs=3) as epool,
                tc.tile_pool(name="fin_sb", bufs=2) as fsb,
            ):
                CQ_NB = NBS // 128   # query chunks per n-block

                def project_q(t):
                    ps_q = auxpool.tile([C, 512], F32, tag="aux", name="ps_q")
                    nc.tensor.matmul(ps_q[32:48, :], lhsT=w_sb[:, WQ1],
                                     rhs=xq_sb[:, ts(t, 512)], start=True, stop=True)
                    nc.tensor.matmul(ps_q[64:80, :], lhsT=w_sb[:, WQ2],
                                     rhs=xq_sb[:, ts(t, 512)], start=True, stop=True)
                    nc.vector.tensor_copy(qq_sb[32:48, ts(t, 512)], ps_q[32:48, :])
                    nc.vector.tensor_copy(qq_sb[64:80, ts(t, 512)], ps_q[64:80, :])

                def project_kv(t):
                    ps_kv = auxpool.tile([C, 512], F32, tag="aux", name="ps_kv")
                    nc.tensor.matmul(ps_kv[32:48, :], lhsT=w_sb[:, WK1],
                                     rhs=xs_sb[:, ts(t, 512)], start=True, stop=True)
                    nc.tensor.matmul(ps_kv[64:80, :], lhsT=w_sb[:, WK2],
                                     rhs=xs_sb[:, ts(t, 512)], start=True, stop=True)
                    nc.vector.tensor_copy(kk_sb[32:48, ts(t, 512)], ps_kv[32:48, :])
                    nc.vector.tensor_copy(kk_sb[64:80, ts(t, 512)], ps_kv[64:80, :])

                def project_vt(m):
                    ps_vt = auxpool.tile([C, 512], F32, tag="aux", name="ps_vt")
                    nc.tensor.matmul(ps_vt[:, 0:HD], lhsT=xs_sb[:, ts(m, 128)],
                                     rhs=w_sb[:, WV], start=True, stop=True)
                    nc.vector.tensor_copy(vTa_sb[:, ds(m * 33, HD)], ps_vt[:, 0:HD])

                def finalize_nb(nb):
                    # transpose av -> [n, 33] (av1/av2 pairs are in distinct
                    # row groups so they can run concurrently), then
                    # normalize by the sums column and combine on DVE
                    psT1 = auxpool.tile([C, CQ_NB * 64], F32, tag="aux")
                    psT2 = auxpool.tile([C, CQ_NB * 64], F32, tag="aux")
                    for cq in range(CQ_NB):
                        gq = nb * CQ_NB + cq
                        nc.tensor.transpose(psT1[:, ds(cq * 64, 33)],
                                            av_sb[0:33, ts(gq, 128)], id_sb[0:33, :])
                        nc.tensor.transpose(psT2[:, ds(cq * 64, 33)],
                                            av_sb[64:97, ts(gq, 128)], id_sb[64:97, :])
                    r1_sb = fsb.tile([C, CQ_NB], F32, tag="r1")
                    r2_sb = fsb.tile([C, CQ_NB], F32, tag="r2")
                    sum1_view = psT1[:, :].rearrange("p (c x) -> p c x", x=64)[:, :, 32:33]
                    sum2_view = psT2[:, :].rearrange("p (c x) -> p c x", x=64)[:, :, 32:33]
                    nc.vector.reciprocal(r1_sb[:, :, None], sum1_view)
                    nc.vector.reciprocal(r2_sb[:, :, None], sum2_view)
                    nc.vector.tensor_scalar_mul(r2_sb[:, :], r2_sb[:, :], -LAMBDA)
                    o1_sb = fsb.tile([C, CQ_NB * HD], F32, tag="o1")
                    o2_sb = fsb.tile([C, CQ_NB * HD], F32, tag="o2")
                    av1t_view = psT1[:, :].rearrange("p (c x) -> p c x", x=64)[:, :, 0:32]
                    av2t_view = psT2[:, :].rearrange("p (c x) -> p c x", x=64)[:, :, 0:32]
                    o1_view = o1_sb[:, :].rearrange("p (c d) -> p c d", d=HD)
                    o2_view = o2_sb[:, :].rearrange("p (c d) -> p c d", d=HD)
                    nc.vector.tensor_tensor(
                        o1_view, av1t_view,
                        r1_sb[:, :, None].to_broadcast((C, CQ_NB, HD)),
                        mybir.AluOpType.mult)
                    nc.vector.tensor_tensor(
                        o2_view, av2t_view,
                        r2_sb[:, :, None].to_broadcast((C, CQ_NB, HD)),
                        mybir.AluOpType.mult)
                    nc.vector.tensor_tensor(
                        out_sb[:, ds(nb * CQ_NB * HD, CQ_NB * HD)],
                        o1_sb[:, :], o2_sb[:, :], mybir.AluOpType.add)

                pending_finalize = None
                N_NB = NQL // NBS
                for nb in range(N_NB):
                    av_ps = avpool.tile([C, NBS], F32, tag="av")
                    if nb == 0:
                        # minimal chain to the first exp: q chunk 0 then the
                        # first k chunk, then the rest of this block's queries
                        project_q(0)
                        project_kv(0)
                        for t in range(1, NBS // 512):
                            project_q(t)
                    for m in range(MC):
                        if nb == 0:
                            project_vt(m)
                            # k chunks 3 iterations ahead of first use, on
                            # the off-beat of the scores they unblock
                            if m % 4 == 1 and (m - 1) // 4 + 1 < NT // 512:
                                project_kv((m - 1) // 4 + 1)
                            # later blocks' query chunks
                            if m % 4 == 3:
                                t = NBS // 512 + (m - 3) // 4
                                if t < NQL // 512:
                                    project_q(t)
                        if m == 2 and pending_finalize is not None:
                            pending_finalize()
                            pending_finalize = None
                        first, last = (m == 0), (m == MC - 1)
                        for j in range(NBS // 512):
                            nsl = ds(nb * NBS + j * 512, 512)
                            # s1 and s2 share one psum tile (adjacent banks):
                            # both row-tiled matmuls are released by the same
                            # ACT read, so the scheduler keeps them adjacent
                            # and they run concurrently in strips 1 and 2.
                            sj_ps = spool.tile([C, 1024], F32, tag="sj")
                            nc.tensor.matmul(sj_ps[:, 0:512],
                                             lhsT=kk_sb[32:48, ts(m, 128)],
                                             rhs=qq_sb[32:48, nsl],
                                             start=True, stop=True)
                            nc.tensor.matmul(sj_ps[:, 512:1024],
                                             lhsT=kk_sb[64:80, ts(m, 128)],
                                             rhs=qq_sb[64:80, nsl],
                                             start=True, stop=True)
                            # one exp over both matrices' scores
                            e_sb = epool.tile([C, 1024], BF16, tag="e")
                            nc.scalar.activation(e_sb[:, :], sj_ps[:, :], Exp,
                                                 scale=SCALE)
                            # av1 (parts 0:33) and av2 (64:97) share psum
                            # banks on disjoint partitions; the sim's group
                            # check is partition-unaware, hence the skip.
                            nc.tensor.matmul(av_ps[0:33, ts(j, 512)],
                                             lhsT=vTa_sb[:, ds(m * 33, 33)],
                                             rhs=e_sb[:, 0:512],
                                             start=first, stop=last,
                                             skip_group_check=True)
                            nc.tensor.matmul(av_ps[64:97, ts(j, 512)],
                                             lhsT=vTa_sb[:, ds(m * 33, 33)],
                                             rhs=e_sb[:, 512:1024],
                                             start=first, stop=last,
                                             skip_group_check=True)
                    # drain the accumulators on DVE + ACT concurrently: ACT
                    # idles at the block transition anyway, and a fast drain
                    # unblocks the next block's AV accumulation (av bufs=1)
                    nc.vector.tensor_copy(av_sb[0:33, ds(nb * NBS, NBS)], av_ps[0:33, :])
                    nc.scalar.copy(av_sb[64:97, ds(nb * NBS, NBS)], av_ps[64:97, :])
                    if nb == N_NB - 1:
                        finalize_nb(nb)
                    else:
                        pending_finalize = (lambda nb=nb: finalize_nb(nb))
                if pending_finalize is not None:
                    pending_finalize()

            nc.sync.dma_start(
                out=out_d[:, :].rearrange("(c p) d -> p c d", p=C),
                in_=out_sb[:, :].rearrange("p (c d) -> p c d", d=HD),
            )
    nc.compile()
    return nc


def make_identity_input():
    ident = np.zeros((C, 33), np.float32)
    ident[0:33, :] = np.eye(33, dtype=np.float32)
    ident[64:97, :] = np.eye(33, dtype=np.float32)
    return ident


def make_in_maps(x, w_qkv):
    """Host-side sharding: subsample, pack per-core inputs."""
    xs = np.ascontiguousarray(x[0][:, ::R, ::R, ::R]).reshape(C, N)
    xs_b = xs.astype(NP_BF16)
    ident = make_identity_input()
    in_maps = []
    for core in range(N_CORES):
        h, half = divmod(core, 2)
        wq = w_qkv[h * 96: h * 96 + 32]       # [32, 128]
        wk = w_qkv[h * 96 + 32: h * 96 + 64]
        wv = w_qkv[h * 96 + 64: h * 96 + 96]
        w = np.empty((C, 96), np.float32)
        w[:, WV] = wv.T
        w[:, WK1] = wk[0:DH].T
        w[:, WK2] = wk[DH:HD].T
        w[:, WQ1] = wq[0:DH].T
        w[:, WQ2] = wq[DH:HD].T
        in_maps.append({
            "xs": xs_b,
            "xq": np.ascontiguousarray(xs_b[:, half * NQ:(half + 1) * NQ]),
            "w": w.astype(NP_BF16),
            "ident": ident,
        })
    return in_maps


_NC_CACHE = {}


def get_nc():
    if "nc" not in _NC_CACHE:
        _NC_CACHE["nc"] = build_nc()
    return _NC_CACHE["nc"]


LAST_RESULTS = None  # BassKernelResults of the most recent kernel() call


def kernel(x, w_qkv, trace=False, **trace_kwargs):
    global LAST_RESULTS
    x = np.asarray(x)
    w_qkv = np.asarray(w_qkv)
    in_maps = make_in_maps(x, w_qkv)
    nc = get_nc()
    res = run_bass_kernel_spmd(nc, in_maps, list(range(N_CORES)),
                               trace=trace, **trace_kwargs)
    LAST_RESULTS = res
    out_hnd = np.empty((HEADS, N, HD), np.float32)
    for core in range(N_CORES):
        h, half = divmod(core, 2)
        out_hnd[h, half * NQ:(half + 1) * NQ, :] = res.results[core]["out"]
    return out_hnd.reshape(1, C, 16, 16, 16)
